# revision 26
# baseline (speedup 1.0000x reference)
"""Trainium2 Bass kernel: multi-head attention w/ additive mask, softmax, dropout.

Reference math (per head h):
    s    = (Q @ K^T) / sqrt(D)                    [S, S]
    t    = s + mask
    P    = softmax(t, axis=-1)
    out  = (P * drop / keep) @ V                   drop ~ bernoulli(key 42)

Device formulation (head h on core h, S^T orientation: k on partitions):
    e0[k,q]  = exp(s^T[k,q])              (ScalarE, PSUM -> SBUF fp16)
    e        = e0 * EM      where EM[k,q]  = exp(mask[q,k])          (VectorE 2x)
    eb       = e0 * EMB     where EMB[k,q] = EM[k,q] * drop[h,q,k]   (VectorE 2x)
    acc[0:8,q] = sum_k ones_aug[k,0:8] * e[k,q]   (softmax sums)
    acc[8+d,q] = sum_k v_aug[k,8+d] * eb[k,q]     (PV path)
    out[q,d]   = acc[8+d,q] / acc[0,q]            (recip via exp(-ln) on ScalarE)

The additive mask is applied multiplicatively post-exp (exp(s+m) = exp(s)exp(m))
because a PSUM-operand tensor add runs at 1x on VectorE while fp16 multiplies
run at 2x; the softmax denominator needs the un-dropped e, so EM and EMB ship
as two precomputed planes of one tensor.

v_aug = [0_{S x 8} | V/keep] and ones_aug = [1_{S x 8} | 0_{S x 120}] are both
128 columns wide so LDWEIGHTS takes the fast-weight-load path, and both
accumulate into one PSUM region (each contributes zeros to the other's rows).
Sums sit at partition 0 because engine accesses need 32-aligned bases.
A burst of dummy matmuls at kernel start runs during the initial DMAs so the
PE_HAM clock gate reaches 2.4 GHz before the real pipeline begins.
"""

import math
import sys

import numpy as np

sys.path.insert(0, "/opt/trn_rl_repo")

B, H, S, D = 1, 8, 1415, 120
S_PAD = 1416  # pad q axis to even count (DVE 2x mode wants even innermost dim)
P = 128
KTILES = (S + P - 1) // P  # 12; last tile has 7 rows
CHUNKS = [(0, 512), (512, 512), (1024, S_PAD - 1024)]  # psum-bank-sized q chunks
P_DROP = 0.7460164712281034
KEEP = 1.0 - P_DROP

# on-chip dtype for matmul operands / exp outputs / masks. float16 keeps
# quantization ~8x below bfloat16 while hitting all the 16-bit fast paths
# (1 cycle/row matmul, FWL weight loads, DVE 2x mode). Value ranges here
# (exp(s+m) <= ~6e3) fit fp16 comfortably.
COMPUTE_DT = "float16"
N_WARMUP_MM = 12  # ~3.5us of dummy matmuls to open the PE_HAM clock gate

_CACHE: dict = {}


def _build_nc_real(compute_dt: str):
    import concourse.bass as bass
    import concourse.tile as tile
    from concourse import mybir

    fp32 = mybir.dt.float32
    cdt = getattr(mybir.dt, compute_dt)

    nc = bass.Bass("TRN2", target_bir_lowering=False, debug=False, num_devices=H)

    qkt = nc.dram_tensor("qkt", [2, D, S_PAD], cdt, kind="ExternalInput").ap()
    # host-preswizzled: v[p, t*128+d] = v_aug[t*128+p, d] so the load is one
    # contiguous [128, 1536] DMA (the strided layout halves DMA efficiency)
    v = nc.dram_tensor("v", [P, KTILES * P], cdt, kind="ExternalInput").ap()
    # mb[0] = EM = exp(mask^T), mb[1] = EMB = EM * drop^T -- one tensor so each
    # k-tile loads both planes in a single DMA
    mb = nc.dram_tensor("mb", [2, S, S_PAD], cdt, kind="ExternalInput").ap()
    out_t = nc.dram_tensor("out_t", [D, S_PAD], fp32, kind="ExternalOutput").ap()

    EXP = mybir.ActivationFunctionType.Exp
    LOG = mybir.ActivationFunctionType.Ln
    COPY = mybir.ActivationFunctionType.Copy
    MULT = mybir.AluOpType.mult

    with tile.TileContext(nc) as tc:
        with (
            tc.tile_pool(name="const", bufs=1) as constp,
            tc.tile_pool(name="io", bufs=4) as iop,
            tc.tile_pool(name="work", bufs=3) as workp,
            tc.tile_pool(name="psa", bufs=2, space="PSUM") as psa,
            tc.tile_pool(name="psb", bufs=1, space="PSUM") as psb,
            tc.tile_pool(name="pacc", bufs=1, space="PSUM") as paccp,
        ):
            # PE warm-up: dummy matmuls on a memset tile, issued while the
            # initial DMAs are in flight (HAM needs ~3.4us of PE busy)
            wtile = constp.tile([P, 512], cdt)
            nc.vector.memset(wtile[:], 0.5)
            ps_warm = psa.tile([P, 1024], fp32, tag="ps_a")
            for _ in range(N_WARMUP_MM):
                nc.tensor.matmul(
                    ps_warm[:, :512], wtile[:, :P], wtile[:], start=True, stop=True
                )

            # Q^T and K^T in one DMA so the first scores matmul waits on a
            # single DMA lane (PE LDWEIGHTS carries at most one sync wait).
            qk_sb = constp.tile([D, 2 * S_PAD], cdt)
            nc.sync.dma_start(
                qk_sb[:].rearrange("p (two s) -> p two s", two=2),
                qkt.rearrange("two p s -> p two s"),
            )
            qt_sb = qk_sb[:, :S_PAD]
            kt_sb = qk_sb[:, S_PAD:]

            # first k-tile's EM/EMB planes early: tile-0 compute only needs
            # qkt + this tile, so don't queue V behind it
            mb_t0 = iop.tile([P, 2 * S_PAD], cdt, tag="mb_t")
            nc.sync.dma_start(
                mb_t0[:].rearrange("p (two s) -> p two s", two=2),
                mb[:, :P, :].rearrange("two p s -> p two s"),
            )

            v_sb = constp.tile([P, KTILES * P], cdt)
            nc.sync.dma_start(v_sb[:], v[:])

            # [1_8 | 0_{120}] weight block: adds softmax sums into rows 0:8
            ones_sb = constp.tile([P, P], cdt)
            nc.vector.memset(ones_sb[:, :8], 1.0)
            nc.vector.memset(ones_sb[:, 8:], 0.0)
            ones_row = constp.tile([1, P], fp32)
            nc.vector.memset(ones_row[:], 1.0)

            # pre-touch the v_sb DMA from the PE so later PV matmuls carry a
            # single (DVE) sync wait (PE LDWEIGHTS holds at most one)
            nc.tensor.matmul(
                ps_warm[:1, 512:513], v_sb[:1, :1], v_sb[:1, :1], start=True, stop=True
            )

            # rows 0:8 = softmax sums, rows 8:128 = out^T accumulator
            ps_acc = paccp.tile([P, S_PAD], fp32, tag="ps_acc")

            # software-pipelined by one k-tile: PE issues next tile's score
            # matmuls before this tile's sums/PV so it never idles on ACT/DVE
            pending = None  # (t, kp, e, eb)
            for t in range(KTILES):
                kp = min(P, S - t * P)
                if t == 0:
                    mb_t = mb_t0
                else:
                    mb_t = iop.tile([P, 2 * S_PAD], cdt, tag="mb_t")
                    nc.sync.dma_start(
                        mb_t[:kp].rearrange("p (two s) -> p two s", two=2),
                        mb[:, t * P : t * P + kp, :].rearrange("two p s -> p two s"),
                    )
                em_t = mb_t[:, :S_PAD]
                emb_t = mb_t[:, S_PAD:]

                e0 = workp.tile([P, S_PAD], cdt, tag="e0")
                # scores in two psum tiles: [0:1024] (2 banks, double-buffered)
                # and [1024:1416] (1 bank); exp evacuates each in one ACT op
                ps_sa = psa.tile([P, 1024], fp32, tag="ps_a")
                for c0 in (0, 512):
                    nc.tensor.matmul(
                        ps_sa[:kp, c0 : c0 + 512],
                        kt_sb[:, t * P : t * P + kp],
                        qt_sb[:, c0 : c0 + 512],
                        start=True,
                        stop=True,
                    )
                nc.scalar.activation(e0[:kp, :1024], ps_sa[:kp, :], EXP)
                ps_sb = psb.tile([P, 392], fp32, tag="ps_b")
                nc.tensor.matmul(
                    ps_sb[:kp, :],
                    kt_sb[:, t * P : t * P + kp],
                    qt_sb[:, 1024:],
                    start=True,
                    stop=True,
                )
                nc.scalar.activation(e0[:kp, 1024:], ps_sb[:kp, :], EXP)

                e = workp.tile([P, S_PAD], cdt, tag="e")
                eb = workp.tile([P, S_PAD], cdt, tag="eb")
                nc.vector.tensor_tensor(e[:kp], e0[:kp], em_t[:kp], op=MULT)
                nc.vector.tensor_tensor(eb[:kp], e0[:kp], emb_t[:kp], op=MULT)

                if pending is not None:
                    _acc_matmuls(nc, pending, ps_acc, ones_sb, v_sb)
                pending = (t, kp, e, eb)
            _acc_matmuls(nc, pending, ps_acc, ones_sb, v_sb)

            # out[q,d] = acc[8+d,q] / sums[q]: reciprocal as exp(-ln(sums)) on
            # ScalarE (VectorE's iterative-divide reciprocal costs ~9us).
            # Partition-broadcast of recip via a K=1 matmul with a ones row.
            lns = constp.tile([1, S_PAD], fp32)
            nc.scalar.activation(lns[:1], ps_acc[:1, :], LOG)
            recip = constp.tile([1, S_PAD], fp32)
            nc.scalar.activation(recip[:1], lns[:1], EXP, scale=-1.0)
            bc_a = psa.tile([P, 1024], fp32, tag="ps_a")
            for c0 in (0, 512):
                nc.tensor.matmul(
                    bc_a[:, c0 : c0 + 512],
                    ones_row[:1, :],
                    recip[:1, c0 : c0 + 512],
                    start=True,
                    stop=True,
                )
            bc_b = psb.tile([P, 392], fp32, tag="ps_b")
            nc.tensor.matmul(
                bc_b[:, :], ones_row[:1, :], recip[:1, 1024:], start=True, stop=True
            )
            recip_bc = constp.tile([P, S_PAD], fp32)
            nc.scalar.activation(recip_bc[:, :1024], bc_a[:, :], COPY)
            nc.scalar.activation(recip_bc[:, 1024:], bc_b[:, :], COPY)
            outf = constp.tile([P, S_PAD], fp32)
            nc.vector.tensor_tensor(
                outf[:, :1024], ps_acc[:, :1024], recip_bc[:, :1024], op=MULT
            )
            nc.sync.dma_start(out_t[:, :1024], outf[8:, :1024])
            nc.vector.tensor_tensor(
                outf[:, 1024:], ps_acc[:, 1024:], recip_bc[:, 1024:], op=MULT
            )
            nc.sync.dma_start(out_t[:, 1024:], outf[8:, 1024:])

    return nc


def _acc_matmuls(nc, pending, ps_acc, ones_sb, v_sb):
    t, kp, e, eb = pending
    last = t == KTILES - 1
    sums = [
        lambda c0=c0, cw=cw: nc.tensor.matmul(
            ps_acc[:, c0 : c0 + cw],
            ones_sb[:kp, :],
            e[:kp, c0 : c0 + cw],
            start=(t == 0),
            stop=False,
        )
        for c0, cw in CHUNKS
    ]
    pvs = [
        lambda c0=c0, cw=cw: nc.tensor.matmul(
            ps_acc[:, c0 : c0 + cw],
            v_sb[:kp, t * P : (t + 1) * P],
            eb[:kp, c0 : c0 + cw],
            start=False,
            stop=(last and c0 == CHUNKS[-1][0]),
        )
        for c0, cw in CHUNKS
    ]
    if last:
        # sums first so the ln/recip chain starts while PV still streams
        for f in sums + pvs:
            f()
    else:
        for s_, p_ in zip(sums, pvs):
            s_()
            p_()


# walrus rejects instructions carrying more than a couple of sync-wait
# commands (the per-ISA-struct sync section is tiny — matmul fits one).
# Spill the excess onto same-engine NOPs emitted just before the instruction;
# the sequencer processes them in order so the semantics are identical.
_WAIT_LIMIT = 1
_SPILL_SKIP = {
    "InstNoOp",
    "InstAllEngineBarrier",
    "InstUnconditionalBranch",
    "InstCompareAndBranch",
    "InstBranchHint",
    "InstEventSemaphore",
    "InstHalt",
}


def _spill_excess_waits(nc):
    from concourse import mybir

    for f in nc.m.functions:
        for blk in f.blocks:
            new = []
            for inst in blk.instructions:
                si = inst.sync_info
                tn = type(inst).__name__
                if (
                    si is not None
                    and si.on_wait
                    and tn not in _SPILL_SKIP
                    and len(si.on_wait) > _WAIT_LIMIT
                ):
                    waits = list(si.on_wait)
                    excess, keep = waits[:-_WAIT_LIMIT], waits[-_WAIT_LIMIT:]
                    for w in excess:
                        new.append(
                            mybir.InstNoOp(
                                name=nc.get_next_instruction_name(),
                                sync_info=mybir.SyncInfo(on_wait=[w], on_update=[]),
                                engine=inst.engine,
                                bass_nofuse=True,
                            )
                        )
                    inst.sync_info = mybir.SyncInfo(
                        on_wait=keep, on_update=si.on_update
                    )
                new.append(inst)
            blk.instructions = new


def _get_nc(compute_dt: str):
    key = ("nc", compute_dt)
    if key not in _CACHE:
        nc = _build_nc_real(compute_dt)
        _spill_excess_waits(nc)
        _CACHE[key] = nc
    return _CACHE[key]


def _drop_mask() -> np.ndarray:
    """Bit-exact reproduction of the reference dropout mask.

    Must run on jax's default backend (neuron via axon in this container) —
    exactly as reference.py does — because the neuron-lowered bernoulli
    produces different bits than the CPU backend."""
    if "drop" not in _CACHE:
        import jax

        keep = jax.random.bernoulli(jax.random.key(42), KEEP, (B, H, S, S))
        _CACHE["drop"] = np.asarray(keep)
    return _CACHE["drop"]


def _np_dt(name: str):
    if name == "float16":
        return np.float16
    if name in ("float32", "float32r"):
        return np.float32
    import ml_dtypes

    return ml_dtypes.bfloat16


def kernel(query, key, value, attn_mask) -> np.ndarray:
    from concourse.bass_utils import run_bass_kernel_spmd

    np_cdt = _np_dt(COMPUTE_DT)

    q = np.asarray(query, np.float32)[0]  # [H, S, D]
    k = np.asarray(key, np.float32)[0]
    v = np.asarray(value, np.float32)[0]
    m = np.asarray(attn_mask, np.float32)[0, 0]  # [S, S]
    drop = _drop_mask()[0]  # [H, S, S] bool

    scale = 1.0 / math.sqrt(D)
    qkt = np.zeros((H, 2, D, S_PAD), np_cdt)
    qkt[:, 0, :, :S] = (q * scale).transpose(0, 2, 1)
    qkt[:, 1, :, :S] = k.transpose(0, 2, 1)

    # v_aug rows (t*128+p) -> sbuf [p, t*128+d]; pad S..KTILES*128 with zeros
    v_aug = np.zeros((H, KTILES * P, P), np.float32)
    v_aug[:, :S, 8:] = v / KEEP
    v_sw = (
        v_aug.reshape(H, KTILES, P, P).transpose(0, 2, 1, 3).reshape(H, P, KTILES * P)
    ).astype(np_cdt)

    em = np.ones((S, S_PAD), np.float32)
    em[:, :S] = np.exp(m.T)
    nc = _get_nc(COMPUTE_DT)

    in_maps = []
    for h in range(H):
        mbh = np.zeros((2, S, S_PAD), np_cdt)
        mbh[0] = em
        mbh[1, :, :S] = (em[:, :S] * drop[h].T)
        in_maps.append(
            {
                "qkt": qkt[h],
                "v": np.ascontiguousarray(v_sw[h]),
                "mb": mbh,
            }
        )

    _CACHE["last_in_maps"] = in_maps
    res = run_bass_kernel_spmd(nc, in_maps, core_ids=list(range(H)))
    out = np.stack([res.results[h]["out_t"][:, :S].T for h in range(H)])
    return out[None].astype(np.float32)  # [1, H, S, D]


# revision 27
# speedup vs baseline: 1.1653x; 1.1653x over previous
"""Trainium2 Bass kernel: multi-head attention w/ additive mask, softmax, dropout.

Reference math (per head h):
    s    = (Q @ K^T) / sqrt(D)                    [S, S]
    t    = s + mask
    P    = softmax(t, axis=-1)
    out  = (P * drop / keep) @ V                   drop ~ bernoulli(key 42)

Device formulation (head h on core h, S^T orientation: k on partitions):
    e0[k,q]  = exp(s^T[k,q])              (ScalarE, PSUM -> SBUF fp16)
    e        = e0 * EM      where EM[k,q]  = exp(mask[q,k])          (VectorE 2x)
    eb       = e0 * EMB     where EMB[k,q] = EM[k,q] * drop[h,q,k]   (VectorE 2x)
    acc[0:8,q] = sum_k ones_aug[k,0:8] * e[k,q]   (softmax sums)
    acc[8+d,q] = sum_k v_aug[k,8+d] * eb[k,q]     (PV path)
    out[q,d]   = acc[8+d,q] / acc[0,q]            (recip via exp(-ln) on ScalarE)

The additive mask is applied multiplicatively post-exp (exp(s+m) = exp(s)exp(m))
because a PSUM-operand tensor add runs at 1x on VectorE while fp16 multiplies
run at 2x; the softmax denominator needs the un-dropped e, so EM and EMB ship
as two precomputed planes of one tensor.

v_aug = [0_{S x 8} | V/keep] and ones_aug = [1_{S x 8} | 0_{S x 120}] are both
128 columns wide so LDWEIGHTS takes the fast-weight-load path, and both
accumulate into one PSUM region (each contributes zeros to the other's rows).
Sums sit at partition 0 because engine accesses need 32-aligned bases.
A burst of dummy matmuls at kernel start runs during the initial DMAs so the
PE_HAM clock gate reaches 2.4 GHz before the real pipeline begins.
"""

import math
import sys

import numpy as np

sys.path.insert(0, "/opt/trn_rl_repo")

B, H, S, D = 1, 8, 1415, 120
S_PAD = 1416  # pad q axis to even count (DVE 2x mode wants even innermost dim)
P = 128
KTILES = (S + P - 1) // P  # 12; last tile has 7 rows
CHUNKS = [(0, 512), (512, 512), (1024, S_PAD - 1024)]  # psum-bank-sized q chunks
P_DROP = 0.7460164712281034
KEEP = 1.0 - P_DROP

# on-chip dtype for matmul operands / exp outputs / masks. float16 keeps
# quantization ~8x below bfloat16 while hitting all the 16-bit fast paths
# (1 cycle/row matmul, FWL weight loads, DVE 2x mode). Value ranges here
# (exp(s+m) <= ~6e3) fit fp16 comfortably.
COMPUTE_DT = "float16"
N_WARMUP_MM = 12  # ~3.5us of dummy matmuls to open the PE_HAM clock gate

_CACHE: dict = {}


def _build_nc_real(compute_dt: str):
    import concourse.bass as bass
    import concourse.tile as tile
    from concourse import mybir

    fp32 = mybir.dt.float32
    cdt = getattr(mybir.dt, compute_dt)

    nc = bass.Bass("TRN2", target_bir_lowering=False, debug=False, num_devices=H)

    qkt = nc.dram_tensor("qkt", [2, D, S_PAD], cdt, kind="ExternalInput").ap()
    # host-preswizzled: v[p, t*128+d] = v_aug[t*128+p, d] so the load is one
    # contiguous [128, 1536] DMA (the strided layout halves DMA efficiency)
    v = nc.dram_tensor("v", [P, KTILES * P], cdt, kind="ExternalInput").ap()
    # mb[0] = EM = exp(mask^T), mb[1] = EMB = EM * drop^T -- one tensor so each
    # k-tile loads both planes in a single DMA
    mb = nc.dram_tensor("mb", [2, S, S_PAD], cdt, kind="ExternalInput").ap()
    out_t = nc.dram_tensor("out_t", [D, S_PAD], fp32, kind="ExternalOutput").ap()

    EXP = mybir.ActivationFunctionType.Exp
    LOG = mybir.ActivationFunctionType.Ln
    COPY = mybir.ActivationFunctionType.Copy
    MULT = mybir.AluOpType.mult

    with tile.TileContext(nc) as tc:
        with (
            tc.tile_pool(name="const", bufs=1) as constp,
            tc.tile_pool(name="io", bufs=4) as iop,
            tc.tile_pool(name="work", bufs=3) as workp,
            tc.tile_pool(name="psa", bufs=2, space="PSUM") as psa,
            tc.tile_pool(name="psb", bufs=1, space="PSUM") as psb,
            tc.tile_pool(name="pacc", bufs=1, space="PSUM") as paccp,
        ):
            # PE warm-up: dummy matmuls on a memset tile, issued while the
            # initial DMAs are in flight (HAM needs ~3.4us of PE busy)
            # separate weight/rhs tiles + alternating PSUM banks: same-tile
            # operands serialize on the SBUF read port, same-bank WAW on drain
            wtile = constp.tile([P, 512], cdt)
            nc.vector.memset(wtile[:], 0.5)
            wtile2 = constp.tile([P, P], cdt)
            nc.vector.memset(wtile2[:], 0.5)
            ps_warm = psa.tile([P, 1024], fp32, tag="ps_a")
            for i in range(N_WARMUP_MM):
                c0 = 512 * (i % 2)
                nc.tensor.matmul(
                    ps_warm[:, c0 : c0 + 512], wtile2[:], wtile[:], start=True, stop=True
                )

            # Q^T and K^T in one DMA so the first scores matmul waits on a
            # single DMA lane (PE LDWEIGHTS carries at most one sync wait).
            qk_sb = constp.tile([D, 2 * S_PAD], cdt)
            nc.sync.dma_start(
                qk_sb[:].rearrange("p (two s) -> p two s", two=2),
                qkt.rearrange("two p s -> p two s"),
            )
            qt_sb = qk_sb[:, :S_PAD]
            kt_sb = qk_sb[:, S_PAD:]

            # first k-tile's EM/EMB planes early: tile-0 compute only needs
            # qkt + this tile, so don't queue V behind it
            mb_t0 = iop.tile([P, 2 * S_PAD], cdt, tag="mb_t")
            nc.sync.dma_start(
                mb_t0[:].rearrange("p (two s) -> p two s", two=2),
                mb[:, :P, :].rearrange("two p s -> p two s"),
            )

            v_sb = constp.tile([P, KTILES * P], cdt)
            nc.sync.dma_start(v_sb[:], v[:])

            # [1_8 | 0_{120}] weight block: adds softmax sums into rows 0:8
            ones_sb = constp.tile([P, P], cdt)
            nc.vector.memset(ones_sb[:, :8], 1.0)
            nc.vector.memset(ones_sb[:, 8:], 0.0)
            ones_row = constp.tile([1, P], fp32)
            nc.vector.memset(ones_row[:], 1.0)

            # pre-touch the v_sb DMA from the PE so later PV matmuls carry a
            # single (DVE) sync wait (PE LDWEIGHTS holds at most one)
            nc.tensor.matmul(
                ps_warm[:1, 512:513], v_sb[:1, :1], v_sb[:1, :1], start=True, stop=True
            )

            # rows 0:8 = softmax sums, rows 8:128 = out^T accumulator
            ps_acc = paccp.tile([P, S_PAD], fp32, tag="ps_acc")

            # software-pipelined by one k-tile: PE issues next tile's score
            # matmuls before this tile's sums/PV so it never idles on ACT/DVE
            pending = None  # (t, kp, e, eb)
            for t in range(KTILES):
                kp = min(P, S - t * P)
                if t == 0:
                    mb_t = mb_t0
                else:
                    mb_t = iop.tile([P, 2 * S_PAD], cdt, tag="mb_t")
                    nc.sync.dma_start(
                        mb_t[:kp].rearrange("p (two s) -> p two s", two=2),
                        mb[:, t * P : t * P + kp, :].rearrange("two p s -> p two s"),
                    )
                em_t = mb_t[:, :S_PAD]
                emb_t = mb_t[:, S_PAD:]

                e0 = workp.tile([P, S_PAD], cdt, tag="e0")
                # scores in two psum tiles: [0:1024] (2 banks, double-buffered)
                # and [1024:1416] (1 bank); exp evacuates each in one ACT op
                ps_sa = psa.tile([P, 1024], fp32, tag="ps_a")
                for c0 in (0, 512):
                    nc.tensor.matmul(
                        ps_sa[:kp, c0 : c0 + 512],
                        kt_sb[:, t * P : t * P + kp],
                        qt_sb[:, c0 : c0 + 512],
                        start=True,
                        stop=True,
                    )
                nc.scalar.activation(e0[:kp, :1024], ps_sa[:kp, :], EXP)
                ps_sb = psb.tile([P, 392], fp32, tag="ps_b")
                nc.tensor.matmul(
                    ps_sb[:kp, :],
                    kt_sb[:, t * P : t * P + kp],
                    qt_sb[:, 1024:],
                    start=True,
                    stop=True,
                )
                nc.scalar.activation(e0[:kp, 1024:], ps_sb[:kp, :], EXP)

                e = workp.tile([P, S_PAD], cdt, tag="e")
                eb = workp.tile([P, S_PAD], cdt, tag="eb")
                nc.vector.tensor_tensor(e[:kp], e0[:kp], em_t[:kp], op=MULT)
                nc.vector.tensor_tensor(eb[:kp], e0[:kp], emb_t[:kp], op=MULT)

                if pending is not None:
                    _acc_matmuls(nc, pending, ps_acc, ones_sb, v_sb)
                pending = (t, kp, e, eb)
            _acc_matmuls(nc, pending, ps_acc, ones_sb, v_sb)

            # out[q,d] = acc[8+d,q] / sums[q]: reciprocal as exp(-ln(sums)) on
            # ScalarE (VectorE's iterative-divide reciprocal costs ~9us).
            # Partition-broadcast of recip via a K=1 matmul with a ones row.
            lns = constp.tile([1, S_PAD], fp32)
            nc.scalar.activation(lns[:1], ps_acc[:1, :], LOG)
            recip = constp.tile([1, S_PAD], fp32)
            nc.scalar.activation(recip[:1], lns[:1], EXP, scale=-1.0)
            bc_a = psa.tile([P, 1024], fp32, tag="ps_a")
            for c0 in (0, 512):
                nc.tensor.matmul(
                    bc_a[:, c0 : c0 + 512],
                    ones_row[:1, :],
                    recip[:1, c0 : c0 + 512],
                    start=True,
                    stop=True,
                )
            bc_b = psb.tile([P, 392], fp32, tag="ps_b")
            nc.tensor.matmul(
                bc_b[:, :], ones_row[:1, :], recip[:1, 1024:], start=True, stop=True
            )
            recip_bc = constp.tile([P, S_PAD], fp32)
            nc.scalar.activation(recip_bc[:, :1024], bc_a[:, :], COPY)
            nc.scalar.activation(recip_bc[:, 1024:], bc_b[:, :], COPY)
            outf = constp.tile([P, S_PAD], fp32)
            nc.vector.tensor_tensor(
                outf[:, :1024], ps_acc[:, :1024], recip_bc[:, :1024], op=MULT
            )
            nc.sync.dma_start(out_t[:, :1024], outf[8:, :1024])
            nc.vector.tensor_tensor(
                outf[:, 1024:], ps_acc[:, 1024:], recip_bc[:, 1024:], op=MULT
            )
            nc.sync.dma_start(out_t[:, 1024:], outf[8:, 1024:])

    return nc


def _acc_matmuls(nc, pending, ps_acc, ones_sb, v_sb):
    t, kp, e, eb = pending
    last = t == KTILES - 1
    sums = [
        lambda c0=c0, cw=cw: nc.tensor.matmul(
            ps_acc[:, c0 : c0 + cw],
            ones_sb[:kp, :],
            e[:kp, c0 : c0 + cw],
            start=(t == 0),
            stop=False,
        )
        for c0, cw in CHUNKS
    ]
    pvs = [
        lambda c0=c0, cw=cw: nc.tensor.matmul(
            ps_acc[:, c0 : c0 + cw],
            v_sb[:kp, t * P : (t + 1) * P],
            eb[:kp, c0 : c0 + cw],
            start=False,
            stop=(last and c0 == CHUNKS[-1][0]),
        )
        for c0, cw in CHUNKS
    ]
    if last:
        # sums first so the ln/recip chain starts while PV still streams
        for f in sums + pvs:
            f()
    else:
        for s_, p_ in zip(sums, pvs):
            s_()
            p_()


# walrus rejects instructions carrying more than a couple of sync-wait
# commands (the per-ISA-struct sync section is tiny — matmul fits one).
# Spill the excess onto same-engine NOPs emitted just before the instruction;
# the sequencer processes them in order so the semantics are identical.
_WAIT_LIMIT = 1
_SPILL_SKIP = {
    "InstNoOp",
    "InstAllEngineBarrier",
    "InstUnconditionalBranch",
    "InstCompareAndBranch",
    "InstBranchHint",
    "InstEventSemaphore",
    "InstHalt",
}


def _spill_excess_waits(nc):
    from concourse import mybir

    for f in nc.m.functions:
        for blk in f.blocks:
            new = []
            for inst in blk.instructions:
                si = inst.sync_info
                tn = type(inst).__name__
                if (
                    si is not None
                    and si.on_wait
                    and tn not in _SPILL_SKIP
                    and len(si.on_wait) > _WAIT_LIMIT
                ):
                    waits = list(si.on_wait)
                    excess, keep = waits[:-_WAIT_LIMIT], waits[-_WAIT_LIMIT:]
                    for w in excess:
                        new.append(
                            mybir.InstNoOp(
                                name=nc.get_next_instruction_name(),
                                sync_info=mybir.SyncInfo(on_wait=[w], on_update=[]),
                                engine=inst.engine,
                                bass_nofuse=True,
                            )
                        )
                    inst.sync_info = mybir.SyncInfo(
                        on_wait=keep, on_update=si.on_update
                    )
                new.append(inst)
            blk.instructions = new


def _get_nc(compute_dt: str):
    key = ("nc", compute_dt)
    if key not in _CACHE:
        nc = _build_nc_real(compute_dt)
        _spill_excess_waits(nc)
        _CACHE[key] = nc
    return _CACHE[key]


def _drop_mask() -> np.ndarray:
    """Bit-exact reproduction of the reference dropout mask.

    Must run on jax's default backend (neuron via axon in this container) —
    exactly as reference.py does — because the neuron-lowered bernoulli
    produces different bits than the CPU backend."""
    if "drop" not in _CACHE:
        import jax

        keep = jax.random.bernoulli(jax.random.key(42), KEEP, (B, H, S, S))
        _CACHE["drop"] = np.asarray(keep)
    return _CACHE["drop"]


def _np_dt(name: str):
    if name == "float16":
        return np.float16
    if name in ("float32", "float32r"):
        return np.float32
    import ml_dtypes

    return ml_dtypes.bfloat16


def kernel(query, key, value, attn_mask) -> np.ndarray:
    from concourse.bass_utils import run_bass_kernel_spmd

    np_cdt = _np_dt(COMPUTE_DT)

    q = np.asarray(query, np.float32)[0]  # [H, S, D]
    k = np.asarray(key, np.float32)[0]
    v = np.asarray(value, np.float32)[0]
    m = np.asarray(attn_mask, np.float32)[0, 0]  # [S, S]
    drop = _drop_mask()[0]  # [H, S, S] bool

    scale = 1.0 / math.sqrt(D)
    qkt = np.zeros((H, 2, D, S_PAD), np_cdt)
    qkt[:, 0, :, :S] = (q * scale).transpose(0, 2, 1)
    qkt[:, 1, :, :S] = k.transpose(0, 2, 1)

    # v_aug rows (t*128+p) -> sbuf [p, t*128+d]; pad S..KTILES*128 with zeros
    v_aug = np.zeros((H, KTILES * P, P), np.float32)
    v_aug[:, :S, 8:] = v / KEEP
    v_sw = (
        v_aug.reshape(H, KTILES, P, P).transpose(0, 2, 1, 3).reshape(H, P, KTILES * P)
    ).astype(np_cdt)

    em = np.ones((S, S_PAD), np.float32)
    em[:, :S] = np.exp(m.T)
    nc = _get_nc(COMPUTE_DT)

    in_maps = []
    for h in range(H):
        mbh = np.zeros((2, S, S_PAD), np_cdt)
        mbh[0] = em
        mbh[1, :, :S] = (em[:, :S] * drop[h].T)
        in_maps.append(
            {
                "qkt": qkt[h],
                "v": np.ascontiguousarray(v_sw[h]),
                "mb": mbh,
            }
        )

    _CACHE["last_in_maps"] = in_maps
    res = run_bass_kernel_spmd(nc, in_maps, core_ids=list(range(H)))
    out = np.stack([res.results[h]["out_t"][:, :S].T for h in range(H)])
    return out[None].astype(np.float32)  # [1, H, S, D]
